# revision 20
# baseline (speedup 1.0000x reference)
"""MultiHeadAttention Trainium2 Bass kernel (8 NeuronCores), v3.

Reference computes (per batch b):
  qp = q @ Wq.T + bq            [S, H*D]   (S=2048, H=8, D=256)
  q_h = qp.reshape(H, S, D)     -- RAW reshape, not split-heads:
        head h <- qp rows [h*256,(h+1)*256), all 2048 cols;
        within head: s2 = ls*8 + g , d  <-> qp[h*256+ls, g*256+d]
  scores_h = q_h @ k_h.T / 16 ; P = softmax ; o_h = P @ v_h
  out[s2, h*256+d] = o_h[s2, d] ;  y = out @ Wo.T + bo

Sharding: core c = (b = c//2, hg = c%2) handles batch b, heads
hg*4..hg*4+4. Within a head we work in the permuted sequence order
s2' = g*256 + ls (softmax is row-wise so a consistent permutation of
rows/cols is exact); host applies the inverse permutation.

v3 design notes (measured on HW):
- f32r matmuls self-load weights (no separate LDWEIGHTS): ~224 ns for
  N=512. bf16/fp8 matmuls pay a SERIAL ldweights (128 cols ~ 107 ns,
  DR 256 cols ~ 213 ns). So everything uses f32r EXCEPT QK^T, where
  fp8e4m3 DoubleRow folds the 256-long contraction into one MM
  (213 LDW + 107 stream = 320/jc vs f32r's 448/jc).
- Q/K projections are written as fp8 (x QA=8) by the DVE bias-add;
  exp() un-scales by 1/(16*QA^2). 2D fp8 tiles + rearranged 3D views
  (the DVE 3D-slice write path corrupts data on HW).
- Output projection is Wo-stationary: yp[o, q] = sum_d WoT[d, oc]
  . o_norm[d, q]; yacc accumulates [o x q']; host un-permutes.
- Emission interleaves QK(ig) with PV(ig-1) per key chunk so the PE
  is not throttled by the exp/activation drain; the next head's K/Q
  projections fill the PV-only steps (keeps HAM warm).
- PSUM: S2 (scores) + A2 (proj q/v + rowsum) + O2 (PV acc) + Y2
  (outproj + proj k) = 8 banks.
"""

import os as _os
import numpy as np

B, S, D, H = 4, 2048, 256, 8
HG = 2            # head groups (cores per batch)
HPG = H // HG     # heads per group = 4
SH = S // H       # seq rows owned by one head = 256
NCORES = 8
QA = 8.0          # fp8 quantization scale for q/k projections
EXP_SCALE = 1.0 / (16.0 * QA * QA)   # undo QA^2, apply 1/sqrt(d_k)

_CACHE = {}


def _build():
    import concourse.bacc as bacc
    import concourse.mybir as mybir
    from concourse.tile import TileContext

    F32 = mybir.dt.float32
    F32R = mybir.dt.float32r
    F8 = mybir.dt.float8e4
    DR = mybir.MatmulPerfMode.DoubleRow
    EXP = mybir.ActivationFunctionType.Exp
    MULT = mybir.AluOpType.mult
    ADD = mybir.AluOpType.add

    nc = bacc.Bacc("TRN2", target_bir_lowering=False)

    SC = HPG * SH  # 1024 seq rows owned by this core

    # ---- DRAM I/O (per-core SPMD) ----
    xqT_d = nc.dram_tensor("xqT", [D, SC], F32R, kind="ExternalInput")
    xkT_d = nc.dram_tensor("xkT", [D, SC], F32R, kind="ExternalInput")
    xvT_d = nc.dram_tensor("xvT", [D, SC], F32R, kind="ExternalInput")
    WqT_d = nc.dram_tensor("WqT", [D, S], F32R, kind="ExternalInput")
    WkT_d = nc.dram_tensor("WkT", [D, S], F32R, kind="ExternalInput")
    WvT_d = nc.dram_tensor("WvT", [D, S], F32R, kind="ExternalInput")
    WoT_d = nc.dram_tensor("WoT", [HPG * D, D], F32R, kind="ExternalInput")
    bqT_d = nc.dram_tensor("bqT", [128, 16], F32, kind="ExternalInput")
    bkT_d = nc.dram_tensor("bkT", [128, 16], F32, kind="ExternalInput")
    bvr_d = nc.dram_tensor("bvr", [1, S], F32, kind="ExternalInput")
    out_d = nc.dram_tensor("part", [D, S], F32, kind="ExternalOutput")

    with TileContext(nc) as tc:
        with nc.allow_low_precision(reason="f32r/fp8 attention"), \
             tc.tile_pool(name="sb", bufs=1) as sb, \
             tc.tile_pool(name="ps", bufs=1, space="PSUM") as ps:

            def sbt(shape, dt, tag, **kw):
                return sb.tile(shape, dt, tag=tag, name=tag, **kw)

            # ---- persistent SBUF tiles ----
            WqT = [sbt([128, S], F32R, f"wq{i}") for i in range(2)]
            WkT = [sbt([128, S], F32R, f"wk{i}") for i in range(2)]
            WvT = [sbt([128, S], F32R, f"wv{i}") for i in range(2)]
            xqT = [sbt([128, SC], F32R, f"xq{i}") for i in range(2)]
            xkT = [sbt([128, SC], F32R, f"xk{i}") for i in range(2)]
            xvT = [sbt([128, SC], F32R, f"xv{i}") for i in range(2)]
            WoT = [sbt([128, D], F32R, f"wo{i}") for i in range(8)]
            bqT = sbt([128, 16], F32, "bqT")
            bkT = sbt([128, 16], F32, "bkT")
            bvr = sbt([1, S], F32, "bvr")
            bvb = sbt([128, S], F32, "bvb")
            # single-slotted projection buffers: head h+1's writes naturally
            # WAR-wait on head h's last reads, which complete a step earlier
            qf8 = [sbt([128, 2 * S], F8, "qf8_0")]
            kf8 = [sbt([128, 2 * S], F8, "kf8_0")]
            qf8_3 = [q[:].rearrange("p (c s) -> p c s", c=2) for q in qf8]
            kf8_3 = [k[:].rearrange("p (c s) -> p c s", c=2) for k in kf8]
            vproj = [[sbt([128, S], F32R, f"vp_{sc}") for sc in range(2)]]
            # 2-ig rotating window for normalized PV output
            o_sb = [sbt([128, 1024], F32R, f"osb{dc}") for dc in range(2)]
            yaccT = [sbt([128, S], F32, f"yacc{oc}") for oc in range(2)]

            # ---- startup DMAs: priority order, round-robin 4 queues ----
            qs = [nc.sync, nc.scalar, nc.gpsimd]
            _qi = [0]

            def dma(dst, src):
                qs[_qi[0] % len(qs)].dma_start(dst, src)
                _qi[0] += 1

            dma(bqT[:], bqT_d[:])
            dma(bkT[:], bkT_d[:])
            dma(bvr[:], bvr_d[:])
            # Q path for head 0 first
            for i in range(2):
                dma(xqT[i][:, 0:SH], xqT_d[i * 128:(i + 1) * 128, 0:SH])
            for q4 in range(4):
                for i in range(2):
                    dma(WqT[i][:, q4 * 512:(q4 + 1) * 512],
                        WqT_d[i * 128:(i + 1) * 128, q4 * 512:(q4 + 1) * 512])
            # K path for head 0
            for i in range(2):
                dma(xkT[i][:, 0:SH], xkT_d[i * 128:(i + 1) * 128, 0:SH])
            for q4 in range(4):
                for i in range(2):
                    dma(WkT[i][:, q4 * 512:(q4 + 1) * 512],
                        WkT_d[i * 128:(i + 1) * 128, q4 * 512:(q4 + 1) * 512])
            # V path for head 0
            for i in range(2):
                dma(xvT[i][:, 0:SH], xvT_d[i * 128:(i + 1) * 128, 0:SH])
            for q4 in range(4):
                for i in range(2):
                    dma(WvT[i][:, q4 * 512:(q4 + 1) * 512],
                        WvT_d[i * 128:(i + 1) * 128, q4 * 512:(q4 + 1) * 512])
            # remaining x columns (heads 1-3), Wo
            for i in range(2):
                dma(xqT[i][:, SH:SC], xqT_d[i * 128:(i + 1) * 128, SH:SC])
                dma(xkT[i][:, SH:SC], xkT_d[i * 128:(i + 1) * 128, SH:SC])
                dma(xvT[i][:, SH:SC], xvT_d[i * 128:(i + 1) * 128, SH:SC])
            for i in range(8):
                dma(WoT[i][:], WoT_d[i * 128:(i + 1) * 128, :])

            nc.gpsimd.partition_broadcast(bvb[:], bvr[:])

            ones_f = sbt([128, 1], F32, "ones_f")
            nc.vector.memset(ones_f[:], 1.0)
            ones = sbt([128, 1], F32R, "ones")
            nc.vector.tensor_copy(ones[:], ones_f[:])

            # HAM pre-warm: dummy matmuls on an (uninitialized) scratch tile
            # keep the PE busy through the startup-DMA window so the clock
            # gate is at 8/8 when the first real matmul issues. Results are
            # never read.
            warm_f = sbt([128, 512], F32, "warm_f")
            nc.vector.memset(warm_f[:], 0.0)
            warm_in = sbt([128, 512], F32R, "warm_in")
            nc.vector.tensor_copy(warm_in[:], warm_f[:])
            for _ in range(36):
                wp = ps.tile([128, 512], F32, tag="S", bufs=2, name="wp")
                nc.tensor.matmul(wp[:], warm_in[:, 0:128], warm_in[:],
                                 start=True, stop=True)

            # ---- emitters ----
            P = {}       # (ig, jc) -> pt tile (transient per head)
            STATE = {}   # ig -> (rs, o_ps pair)

            def emit_qchunk(t, ec):
                pq = ps.tile([128, 512], F32, tag="A", bufs=2, name="pq")
                for dc in range(2):
                    nc.tensor.matmul(
                        pq[:, :SH],
                        WqT[dc][:, ec * 128:(ec + 1) * 128],
                        xqT[dc][:, t * SH:(t + 1) * SH],
                        start=(dc == 0), stop=(dc == 1))
                g, dct = divmod(ec, 2)
                nc.vector.tensor_scalar(
                    out=qf8[0][:, dct * S + g * SH:dct * S + (g + 1) * SH],
                    in0=pq[:, :SH], scalar1=QA,
                    scalar2=bqT[:, ec:ec + 1], op0=MULT, op1=ADD)

            def emit_kchunk(t, ec):
                pk = ps.tile([128, 512], F32, tag="Y", bufs=2, name="pk")
                for dc in range(2):
                    nc.tensor.matmul(
                        pk[:, :SH],
                        WkT[dc][:, ec * 128:(ec + 1) * 128],
                        xkT[dc][:, t * SH:(t + 1) * SH],
                        start=(dc == 0), stop=(dc == 1))
                g, dct = divmod(ec, 2)
                nc.vector.tensor_scalar(
                    out=kf8[0][:, dct * S + g * SH:dct * S + (g + 1) * SH],
                    in0=pk[:, :SH], scalar1=QA,
                    scalar2=bkT[:, ec:ec + 1], op0=MULT, op1=ADD)

            def emit_vchunk(t, i):
                sc, ng = divmod(i, 4)
                pv = ps.tile([128, 512], F32, tag="A", bufs=2, name="pv")
                for dc in range(2):
                    nc.tensor.matmul(
                        pv[:],
                        xvT[dc][:, t * SH + sc * 128:t * SH + (sc + 1) * 128],
                        WvT[dc][:, ng * 512:(ng + 1) * 512],
                        start=(dc == 0), stop=(dc == 1))
                nc.vector.tensor_add(
                    vproj[0][sc][:, ng * 512:(ng + 1) * 512], pv[:],
                    bvb[:, ng * 512:(ng + 1) * 512])

            def emit_qk(h, ig, jc):
                sp = ps.tile([128, 512], F32, tag="S", bufs=2, name="sp")
                nc.tensor.matmul(
                    sp[:],
                    kf8_3[0][:, :, jc * 128:(jc + 1) * 128],
                    qf8_3[0][:, :, ig * 512:(ig + 1) * 512],
                    start=True, stop=True, perf_mode=DR)
                pt = sb.tile([128, 512], F32R, tag="p", bufs=20, name="pt")
                nc.scalar.activation(pt[:], sp[:], EXP, scale=EXP_SCALE)
                P[(ig, jc)] = pt

            def emit_pv(h, ig, jc):
                if jc == 0:
                    rs = ps.tile([128, 512], F32, tag="A", bufs=2, name="rs")
                    o_ps = [ps.tile([128, 512], F32, tag="O", bufs=2,
                                    name=f"o{dc}") for dc in range(2)]
                    STATE[ig] = (rs, o_ps)
                rs, o_ps = STATE[ig]
                g, half = divmod(jc, 2)
                pt = P.pop((ig, jc))
                nc.tensor.matmul(
                    rs[0:1, :], ones[:], pt[:],
                    start=(jc == 0), stop=(jc == 15), skip_group_check=True)
                for dc in range(2):
                    nc.tensor.matmul(
                        o_ps[dc][:],
                        vproj[0][half][:, g * SH + dc * 128:g * SH + (dc + 1) * 128],
                        pt[:],
                        start=(jc == 0), stop=(jc == 15), skip_group_check=True)

            def emit_norm(h, ig):
                rs, o_ps = STATE.pop(ig)
                rcp = sb.tile([1, 512], F32, tag="rcp", bufs=2, name="rcp")
                nc.vector.reciprocal_approx_fast(out=rcp[:], in_=rs[0:1, :])
                bc = sb.tile([128, 512], F32, tag="bc", bufs=2, name="bc")
                nc.gpsimd.partition_broadcast(bc[:], rcp[:])
                w = (ig % 2) * 512
                for dc in range(2):
                    nc.vector.tensor_mul(
                        o_sb[dc][:, w:w + 512], o_ps[dc][:], bc[:])

            def emit_outproj(h, ig, ocs=(0, 1)):
                icol = ig * 512
                w = (ig % 2) * 512
                for oc in ocs:
                    yp = ps.tile([128, 512], F32, tag="Y", bufs=2, name="yp")
                    for dc in range(2):
                        nc.tensor.matmul(
                            yp[:],
                            WoT[h * 2 + dc][:, oc * 128:(oc + 1) * 128],
                            o_sb[dc][:, w:w + 512],
                            start=(dc == 0), stop=(dc == 1))
                    if h == 0:
                        nc.vector.tensor_copy(yaccT[oc][:, icol:icol + 512], yp[:])
                    else:
                        nc.vector.tensor_add(
                            yaccT[oc][:, icol:icol + 512],
                            yaccT[oc][:, icol:icol + 512], yp[:])
                        if h == HPG - 1:
                            qd = nc.sync if oc == 0 else nc.scalar
                            qd.dma_start(
                                out_d[oc * 128:(oc + 1) * 128, icol:icol + 512],
                                yaccT[oc][:, icol:icol + 512])

            # ---- head-0 projections (DMA-gated warmup) ----
            for ec in range(4):
                emit_qchunk(0, ec)
            for ec in range(16):
                emit_kchunk(0, ec)
            for ec in range(4, 16):
                emit_qchunk(0, ec)

            # ---- pipelined head loop ----
            # S0: QK(ig0) + V-proj fillers
            # S1-3: QK(ig) interleaved with PV(ig-1) [+ outproj(ig-2) @jc6]
            # S4: PV(ig3) + K-proj(h+1) fillers [+ outproj(ig2) @jc6]
            # S5: Q-proj(h+1) + outproj(ig3)
            for h in range(HPG):
                # S0
                for jc in range(16):
                    if jc % 2 == 0:
                        emit_vchunk(h, jc // 2)
                    emit_qk(h, 0, jc)
                # S1..S3
                for ig in range(1, 4):
                    for jc in range(16):
                        emit_qk(h, ig, jc)
                        if ig >= 2 and jc == 3:
                            emit_outproj(h, ig - 2, (0,))
                        if ig >= 2 and jc == 9:
                            emit_outproj(h, ig - 2, (1,))
                        emit_pv(h, ig - 1, jc)
                    emit_norm(h, ig - 1)
                # S4
                for jc in range(16):
                    if h < HPG - 1:
                        emit_kchunk(h + 1, jc)
                    if jc == 3:
                        emit_outproj(h, 2, (0,))
                    if jc == 9:
                        emit_outproj(h, 2, (1,))
                    emit_pv(h, 3, jc)
                emit_norm(h, 3)
                # S5
                if h < HPG - 1:
                    for ec in range(16):
                        if ec == 4:
                            emit_outproj(h, 3)
                        emit_qchunk(h + 1, ec)
                else:
                    emit_outproj(h, 3)

    nc.finalize()
    return nc


def _get_nc():
    if "nc" not in _CACHE:
        _CACHE["nc"] = _build()
    return _CACHE["nc"]


def _prep_inputs(query, key, values, Wq, bq, Wk, bk, Wv, bv, Wo, bo):
    f32 = np.float32
    query = np.asarray(query, f32)
    key = np.asarray(key, f32)
    values = np.asarray(values, f32)
    WqT = np.ascontiguousarray(np.asarray(Wq, f32).T)
    WkT = np.ascontiguousarray(np.asarray(Wk, f32).T)
    WvT = np.ascontiguousarray(np.asarray(Wv, f32).T)
    WoT = np.ascontiguousarray(np.asarray(Wo, f32).T)
    bqT = np.ascontiguousarray((np.asarray(bq, f32) * QA).reshape(16, 128).T)
    bkT = np.ascontiguousarray((np.asarray(bk, f32) * QA).reshape(16, 128).T)
    bvr = np.ascontiguousarray(np.asarray(bv, f32).reshape(1, S))

    in_maps = []
    for c in range(NCORES):
        b, hg = divmod(c, HG)
        rows = slice(hg * HPG * SH, (hg + 1) * HPG * SH)
        in_maps.append({
            "xqT": np.ascontiguousarray(query[b, rows, :].T),
            "xkT": np.ascontiguousarray(key[b, rows, :].T),
            "xvT": np.ascontiguousarray(values[b, rows, :].T),
            "WqT": WqT, "WkT": WkT, "WvT": WvT,
            "WoT": np.ascontiguousarray(WoT[hg * HPG * D:(hg + 1) * HPG * D, :]),
            "bqT": bqT, "bkT": bkT, "bvr": bvr,
        })
    return in_maps


def _enable_tracing_shims():
    """Best-effort: make trace=True survivable in environments where the
    image's antenv lacks axon_hooks and artifact upload has no network."""
    import sys
    import types
    try:
        import antenv.axon_hooks  # noqa: F401
    except Exception:
        try:
            from trn_agent_boot.trn_boot import _ntff_profile_via_ctypes
            hook = _ntff_profile_via_ctypes("/opt/axon/libaxon_pjrt.so")
            mod = types.ModuleType("antenv.axon_hooks")
            mod.get_axon_ntff_profile_hook = lambda: hook
            mod.set_axon_ntff_profile_hook = lambda h: None
            sys.modules["antenv.axon_hooks"] = mod
            import antenv
            antenv.axon_hooks = mod
        except Exception:
            pass
    try:
        import concourse.bass_utils as bu
        from concourse._compat import FishPath
        FishPath.bucket_root()
    except Exception:
        try:
            bu.upload_artifacts = lambda tmpdir: f"local://{tmpdir}"
        except Exception:
            pass


def kernel(**inputs):
    import os
    from concourse.bass_utils import run_bass_kernel_spmd

    nc = _get_nc()
    in_maps = _prep_inputs(**inputs)
    trace = bool(int(os.environ.get("KERNEL_TRACE", "0")))
    if trace or os.environ.get("BASS_TRACE"):
        _enable_tracing_shims()
    res = run_bass_kernel_spmd(nc, in_maps, core_ids=list(range(NCORES)),
                               trace=trace)
    _CACHE["last_result"] = res

    bo = np.asarray(inputs["bo"], np.float32)
    out = np.empty((B, S, D), np.float32)
    for b in range(B):
        # part[o, q'] with q' = g*256 + ls ; true s2 = ls*8 + g
        p0 = res.results[2 * b]["part"].reshape(D, 8, SH)
        p1 = res.results[2 * b + 1]["part"].reshape(D, 8, SH)
        y = (p0 + p1).transpose(2, 1, 0).reshape(S, D)
        out[b] = y + bo
    return out


# revision 21
# speedup vs baseline: 1.0112x; 1.0112x over previous
"""MultiHeadAttention Trainium2 Bass kernel (8 NeuronCores), v3.

Reference computes (per batch b):
  qp = q @ Wq.T + bq            [S, H*D]   (S=2048, H=8, D=256)
  q_h = qp.reshape(H, S, D)     -- RAW reshape, not split-heads:
        head h <- qp rows [h*256,(h+1)*256), all 2048 cols;
        within head: s2 = ls*8 + g , d  <-> qp[h*256+ls, g*256+d]
  scores_h = q_h @ k_h.T / 16 ; P = softmax ; o_h = P @ v_h
  out[s2, h*256+d] = o_h[s2, d] ;  y = out @ Wo.T + bo

Sharding: core c = (b = c//2, hg = c%2) handles batch b, heads
hg*4..hg*4+4. Within a head we work in the permuted sequence order
s2' = g*256 + ls (softmax is row-wise so a consistent permutation of
rows/cols is exact); host applies the inverse permutation.

v3 design notes (measured on HW):
- f32r matmuls self-load weights (no separate LDWEIGHTS): ~224 ns for
  N=512. bf16/fp8 matmuls pay a SERIAL ldweights (128 cols ~ 107 ns,
  DR 256 cols ~ 213 ns). So everything uses f32r EXCEPT QK^T, where
  fp8e4m3 DoubleRow folds the 256-long contraction into one MM
  (213 LDW + 107 stream = 320/jc vs f32r's 448/jc).
- Q/K projections are written as fp8 (x QA=8) by the DVE bias-add;
  exp() un-scales by 1/(16*QA^2). 2D fp8 tiles + rearranged 3D views
  (the DVE 3D-slice write path corrupts data on HW).
- Output projection is Wo-stationary: yp[o, q] = sum_d WoT[d, oc]
  . o_norm[d, q]; yacc accumulates [o x q']; host un-permutes.
- Emission interleaves QK(ig) with PV(ig-1) per key chunk so the PE
  is not throttled by the exp/activation drain; the next head's K/Q
  projections fill the PV-only steps (keeps HAM warm).
- PSUM: S2 (scores) + A2 (proj q/v + rowsum) + O2 (PV acc) + Y2
  (outproj + proj k) = 8 banks.
"""

import os as _os
import numpy as np

B, S, D, H = 4, 2048, 256, 8
HG = 2            # head groups (cores per batch)
HPG = H // HG     # heads per group = 4
SH = S // H       # seq rows owned by one head = 256
NCORES = 8
QA = 8.0          # fp8 quantization scale for q/k projections
EXP_SCALE = 1.0 / (16.0 * QA * QA)   # undo QA^2, apply 1/sqrt(d_k)

_CACHE = {}


def _build():
    import concourse.bacc as bacc
    import concourse.mybir as mybir
    from concourse.tile import TileContext

    F32 = mybir.dt.float32
    F32R = mybir.dt.float32r
    F8 = mybir.dt.float8e4
    DR = mybir.MatmulPerfMode.DoubleRow
    EXP = mybir.ActivationFunctionType.Exp
    MULT = mybir.AluOpType.mult
    ADD = mybir.AluOpType.add

    nc = bacc.Bacc("TRN2", target_bir_lowering=False)

    SC = HPG * SH  # 1024 seq rows owned by this core

    # ---- DRAM I/O (per-core SPMD) ----
    xqT_d = nc.dram_tensor("xqT", [D, SC], F32R, kind="ExternalInput")
    xkT_d = nc.dram_tensor("xkT", [D, SC], F32R, kind="ExternalInput")
    xvT_d = nc.dram_tensor("xvT", [D, SC], F32R, kind="ExternalInput")
    WqT_d = nc.dram_tensor("WqT", [D, S], F32R, kind="ExternalInput")
    WkT_d = nc.dram_tensor("WkT", [D, S], F32R, kind="ExternalInput")
    WvT_d = nc.dram_tensor("WvT", [D, S], F32R, kind="ExternalInput")
    WoT_d = nc.dram_tensor("WoT", [HPG * D, D], F32R, kind="ExternalInput")
    bqT_d = nc.dram_tensor("bqT", [128, 16], F32, kind="ExternalInput")
    bkT_d = nc.dram_tensor("bkT", [128, 16], F32, kind="ExternalInput")
    bvr_d = nc.dram_tensor("bvr", [1, S], F32, kind="ExternalInput")
    out_d = nc.dram_tensor("part", [D, S], F32, kind="ExternalOutput")

    with TileContext(nc) as tc:
        with nc.allow_low_precision(reason="f32r/fp8 attention"), \
             tc.tile_pool(name="sb", bufs=1) as sb, \
             tc.tile_pool(name="ps", bufs=1, space="PSUM") as ps:

            def sbt(shape, dt, tag, **kw):
                return sb.tile(shape, dt, tag=tag, name=tag, **kw)

            # ---- persistent SBUF tiles ----
            WqT = [sbt([128, S], F32R, f"wq{i}") for i in range(2)]
            WkT = [sbt([128, S], F32R, f"wk{i}") for i in range(2)]
            WvT = [sbt([128, S], F32R, f"wv{i}") for i in range(2)]
            xqT = [sbt([128, SC], F32R, f"xq{i}") for i in range(2)]
            xkT = [sbt([128, SC], F32R, f"xk{i}") for i in range(2)]
            xvT = [sbt([128, SC], F32R, f"xv{i}") for i in range(2)]
            WoT = [sbt([128, D], F32R, f"wo{i}") for i in range(8)]
            bqT = sbt([128, 16], F32, "bqT")
            bkT = sbt([128, 16], F32, "bkT")
            bvr = sbt([1, S], F32, "bvr")
            bvb = sbt([128, S], F32, "bvb")
            # single-slotted projection buffers: head h+1's writes naturally
            # WAR-wait on head h's last reads, which complete a step earlier
            qf8 = [sbt([128, 2 * S], F8, "qf8_0")]
            kf8 = [sbt([128, 2 * S], F8, "kf8_0")]
            qf8_3 = [q[:].rearrange("p (c s) -> p c s", c=2) for q in qf8]
            kf8_3 = [k[:].rearrange("p (c s) -> p c s", c=2) for k in kf8]
            vproj = [[sbt([128, S], F32R, f"vp_{sc}") for sc in range(2)]]
            # 2-ig rotating window for normalized PV output
            o_sb = [sbt([128, 1024], F32R, f"osb{dc}") for dc in range(2)]
            yaccT = [sbt([128, S], F32, f"yacc{oc}") for oc in range(2)]

            # ---- startup DMAs: priority order, round-robin 4 queues ----
            qs = [nc.sync, nc.scalar]
            _qi = [0]

            def dma(dst, src):
                qs[_qi[0] % len(qs)].dma_start(dst, src)
                _qi[0] += 1

            dma(bqT[:], bqT_d[:])
            dma(bkT[:], bkT_d[:])
            dma(bvr[:], bvr_d[:])
            # Q path for head 0 first
            for i in range(2):
                dma(xqT[i][:, 0:SH], xqT_d[i * 128:(i + 1) * 128, 0:SH])
            for q4 in range(4):
                for i in range(2):
                    dma(WqT[i][:, q4 * 512:(q4 + 1) * 512],
                        WqT_d[i * 128:(i + 1) * 128, q4 * 512:(q4 + 1) * 512])
            # K path for head 0
            for i in range(2):
                dma(xkT[i][:, 0:SH], xkT_d[i * 128:(i + 1) * 128, 0:SH])
            for q4 in range(4):
                for i in range(2):
                    dma(WkT[i][:, q4 * 512:(q4 + 1) * 512],
                        WkT_d[i * 128:(i + 1) * 128, q4 * 512:(q4 + 1) * 512])
            # V path for head 0
            for i in range(2):
                dma(xvT[i][:, 0:SH], xvT_d[i * 128:(i + 1) * 128, 0:SH])
            for q4 in range(4):
                for i in range(2):
                    dma(WvT[i][:, q4 * 512:(q4 + 1) * 512],
                        WvT_d[i * 128:(i + 1) * 128, q4 * 512:(q4 + 1) * 512])
            # remaining x columns (heads 1-3), Wo
            for i in range(2):
                dma(xqT[i][:, SH:SC], xqT_d[i * 128:(i + 1) * 128, SH:SC])
                dma(xkT[i][:, SH:SC], xkT_d[i * 128:(i + 1) * 128, SH:SC])
                dma(xvT[i][:, SH:SC], xvT_d[i * 128:(i + 1) * 128, SH:SC])
            for i in range(8):
                dma(WoT[i][:], WoT_d[i * 128:(i + 1) * 128, :])

            nc.gpsimd.partition_broadcast(bvb[:], bvr[:])

            ones_f = sbt([128, 1], F32, "ones_f")
            nc.vector.memset(ones_f[:], 1.0)
            ones = sbt([128, 1], F32R, "ones")
            nc.vector.tensor_copy(ones[:], ones_f[:])

            # HAM pre-warm: dummy matmuls on an (uninitialized) scratch tile
            # keep the PE busy through the startup-DMA window so the clock
            # gate is at 8/8 when the first real matmul issues. Results are
            # never read.
            warm_f = sbt([128, 512], F32, "warm_f")
            nc.vector.memset(warm_f[:], 0.0)
            warm_in = sbt([128, 512], F32R, "warm_in")
            nc.vector.tensor_copy(warm_in[:], warm_f[:])
            for _ in range(36):
                wp = ps.tile([128, 512], F32, tag="S", bufs=2, name="wp")
                nc.tensor.matmul(wp[:], warm_in[:, 0:128], warm_in[:],
                                 start=True, stop=True)

            # ---- emitters ----
            P = {}       # (ig, jc) -> pt tile (transient per head)
            STATE = {}   # ig -> (rs, o_ps pair)

            def emit_qchunk(t, ec):
                pq = ps.tile([128, 512], F32, tag="A", bufs=2, name="pq")
                for dc in range(2):
                    nc.tensor.matmul(
                        pq[:, :SH],
                        WqT[dc][:, ec * 128:(ec + 1) * 128],
                        xqT[dc][:, t * SH:(t + 1) * SH],
                        start=(dc == 0), stop=(dc == 1))
                g, dct = divmod(ec, 2)
                nc.vector.tensor_scalar(
                    out=qf8[0][:, dct * S + g * SH:dct * S + (g + 1) * SH],
                    in0=pq[:, :SH], scalar1=QA,
                    scalar2=bqT[:, ec:ec + 1], op0=MULT, op1=ADD)

            def emit_kchunk(t, ec):
                pk = ps.tile([128, 512], F32, tag="Y", bufs=2, name="pk")
                for dc in range(2):
                    nc.tensor.matmul(
                        pk[:, :SH],
                        WkT[dc][:, ec * 128:(ec + 1) * 128],
                        xkT[dc][:, t * SH:(t + 1) * SH],
                        start=(dc == 0), stop=(dc == 1))
                g, dct = divmod(ec, 2)
                nc.vector.tensor_scalar(
                    out=kf8[0][:, dct * S + g * SH:dct * S + (g + 1) * SH],
                    in0=pk[:, :SH], scalar1=QA,
                    scalar2=bkT[:, ec:ec + 1], op0=MULT, op1=ADD)

            def emit_vchunk(t, i):
                sc, ng = divmod(i, 4)
                pv = ps.tile([128, 512], F32, tag="A", bufs=2, name="pv")
                for dc in range(2):
                    nc.tensor.matmul(
                        pv[:],
                        xvT[dc][:, t * SH + sc * 128:t * SH + (sc + 1) * 128],
                        WvT[dc][:, ng * 512:(ng + 1) * 512],
                        start=(dc == 0), stop=(dc == 1))
                nc.vector.tensor_add(
                    vproj[0][sc][:, ng * 512:(ng + 1) * 512], pv[:],
                    bvb[:, ng * 512:(ng + 1) * 512])

            def emit_qk(h, ig, jc):
                sp = ps.tile([128, 512], F32, tag="S", bufs=2, name="sp")
                nc.tensor.matmul(
                    sp[:],
                    kf8_3[0][:, :, jc * 128:(jc + 1) * 128],
                    qf8_3[0][:, :, ig * 512:(ig + 1) * 512],
                    start=True, stop=True, perf_mode=DR)
                pt = sb.tile([128, 512], F32R, tag="p", bufs=20, name="pt")
                nc.scalar.activation(pt[:], sp[:], EXP, scale=EXP_SCALE)
                P[(ig, jc)] = pt

            def emit_pv(h, ig, jc):
                if jc == 0:
                    rs = ps.tile([128, 512], F32, tag="A", bufs=2, name="rs")
                    o_ps = [ps.tile([128, 512], F32, tag="O", bufs=2,
                                    name=f"o{dc}") for dc in range(2)]
                    STATE[ig] = (rs, o_ps)
                rs, o_ps = STATE[ig]
                g, half = divmod(jc, 2)
                pt = P.pop((ig, jc))
                nc.tensor.matmul(
                    rs[0:1, :], ones[:], pt[:],
                    start=(jc == 0), stop=(jc == 15), skip_group_check=True)
                for dc in range(2):
                    nc.tensor.matmul(
                        o_ps[dc][:],
                        vproj[0][half][:, g * SH + dc * 128:g * SH + (dc + 1) * 128],
                        pt[:],
                        start=(jc == 0), stop=(jc == 15), skip_group_check=True)

            def emit_norm(h, ig):
                rs, o_ps = STATE.pop(ig)
                rcp = sb.tile([1, 512], F32, tag="rcp", bufs=2, name="rcp")
                nc.vector.reciprocal_approx_fast(out=rcp[:], in_=rs[0:1, :])
                bc = sb.tile([128, 512], F32, tag="bc", bufs=2, name="bc")
                nc.gpsimd.partition_broadcast(bc[:], rcp[:])
                w = (ig % 2) * 512
                for dc in range(2):
                    nc.vector.tensor_mul(
                        o_sb[dc][:, w:w + 512], o_ps[dc][:], bc[:])

            def emit_outproj(h, ig, ocs=(0, 1)):
                icol = ig * 512
                w = (ig % 2) * 512
                for oc in ocs:
                    yp = ps.tile([128, 512], F32, tag="Y", bufs=2, name="yp")
                    for dc in range(2):
                        nc.tensor.matmul(
                            yp[:],
                            WoT[h * 2 + dc][:, oc * 128:(oc + 1) * 128],
                            o_sb[dc][:, w:w + 512],
                            start=(dc == 0), stop=(dc == 1))
                    if h == 0:
                        nc.vector.tensor_copy(yaccT[oc][:, icol:icol + 512], yp[:])
                    else:
                        nc.vector.tensor_add(
                            yaccT[oc][:, icol:icol + 512],
                            yaccT[oc][:, icol:icol + 512], yp[:])
                        if h == HPG - 1:
                            qd = nc.sync if oc == 0 else nc.scalar
                            qd.dma_start(
                                out_d[oc * 128:(oc + 1) * 128, icol:icol + 512],
                                yaccT[oc][:, icol:icol + 512])

            # ---- head-0 projections (DMA-gated warmup) ----
            for ec in range(4):
                emit_qchunk(0, ec)
            for ec in range(16):
                emit_kchunk(0, ec)
            for ec in range(4, 16):
                emit_qchunk(0, ec)

            # ---- pipelined head loop ----
            # S0: QK(ig0) + V-proj fillers
            # S1-3: QK(ig) interleaved with PV(ig-1) [+ outproj(ig-2) @jc6]
            # S4: PV(ig3) + K-proj(h+1) fillers [+ outproj(ig2) @jc6]
            # S5: Q-proj(h+1) + outproj(ig3)
            for h in range(HPG):
                # S0
                for jc in range(16):
                    if jc % 2 == 0:
                        emit_vchunk(h, jc // 2)
                    emit_qk(h, 0, jc)
                # S1..S3
                for ig in range(1, 4):
                    for jc in range(16):
                        emit_qk(h, ig, jc)
                        if ig >= 2 and jc == 3:
                            emit_outproj(h, ig - 2, (0,))
                        if ig >= 2 and jc == 9:
                            emit_outproj(h, ig - 2, (1,))
                        emit_pv(h, ig - 1, jc)
                    emit_norm(h, ig - 1)
                # S4
                for jc in range(16):
                    if h < HPG - 1:
                        emit_kchunk(h + 1, jc)
                    if jc == 3:
                        emit_outproj(h, 2, (0,))
                    if jc == 9:
                        emit_outproj(h, 2, (1,))
                    emit_pv(h, 3, jc)
                emit_norm(h, 3)
                # S5
                if h < HPG - 1:
                    for ec in range(16):
                        if ec == 4:
                            emit_outproj(h, 3)
                        emit_qchunk(h + 1, ec)
                else:
                    emit_outproj(h, 3)

    nc.finalize()
    return nc


def _get_nc():
    if "nc" not in _CACHE:
        _CACHE["nc"] = _build()
    return _CACHE["nc"]


def _prep_inputs(query, key, values, Wq, bq, Wk, bk, Wv, bv, Wo, bo):
    f32 = np.float32
    query = np.asarray(query, f32)
    key = np.asarray(key, f32)
    values = np.asarray(values, f32)
    WqT = np.ascontiguousarray(np.asarray(Wq, f32).T)
    WkT = np.ascontiguousarray(np.asarray(Wk, f32).T)
    WvT = np.ascontiguousarray(np.asarray(Wv, f32).T)
    WoT = np.ascontiguousarray(np.asarray(Wo, f32).T)
    bqT = np.ascontiguousarray((np.asarray(bq, f32) * QA).reshape(16, 128).T)
    bkT = np.ascontiguousarray((np.asarray(bk, f32) * QA).reshape(16, 128).T)
    bvr = np.ascontiguousarray(np.asarray(bv, f32).reshape(1, S))

    in_maps = []
    for c in range(NCORES):
        b, hg = divmod(c, HG)
        rows = slice(hg * HPG * SH, (hg + 1) * HPG * SH)
        in_maps.append({
            "xqT": np.ascontiguousarray(query[b, rows, :].T),
            "xkT": np.ascontiguousarray(key[b, rows, :].T),
            "xvT": np.ascontiguousarray(values[b, rows, :].T),
            "WqT": WqT, "WkT": WkT, "WvT": WvT,
            "WoT": np.ascontiguousarray(WoT[hg * HPG * D:(hg + 1) * HPG * D, :]),
            "bqT": bqT, "bkT": bkT, "bvr": bvr,
        })
    return in_maps


def _enable_tracing_shims():
    """Best-effort: make trace=True survivable in environments where the
    image's antenv lacks axon_hooks and artifact upload has no network."""
    import sys
    import types
    try:
        import antenv.axon_hooks  # noqa: F401
    except Exception:
        try:
            from trn_agent_boot.trn_boot import _ntff_profile_via_ctypes
            hook = _ntff_profile_via_ctypes("/opt/axon/libaxon_pjrt.so")
            mod = types.ModuleType("antenv.axon_hooks")
            mod.get_axon_ntff_profile_hook = lambda: hook
            mod.set_axon_ntff_profile_hook = lambda h: None
            sys.modules["antenv.axon_hooks"] = mod
            import antenv
            antenv.axon_hooks = mod
        except Exception:
            pass
    try:
        import concourse.bass_utils as bu
        from concourse._compat import FishPath
        FishPath.bucket_root()
    except Exception:
        try:
            bu.upload_artifacts = lambda tmpdir: f"local://{tmpdir}"
        except Exception:
            pass


def kernel(**inputs):
    import os
    from concourse.bass_utils import run_bass_kernel_spmd

    nc = _get_nc()
    in_maps = _prep_inputs(**inputs)
    trace = bool(int(os.environ.get("KERNEL_TRACE", "0")))
    if trace or os.environ.get("BASS_TRACE"):
        _enable_tracing_shims()
    res = run_bass_kernel_spmd(nc, in_maps, core_ids=list(range(NCORES)),
                               trace=trace)
    _CACHE["last_result"] = res

    bo = np.asarray(inputs["bo"], np.float32)
    out = np.empty((B, S, D), np.float32)
    for b in range(B):
        # part[o, q'] with q' = g*256 + ls ; true s2 = ls*8 + g
        p0 = res.results[2 * b]["part"].reshape(D, 8, SH)
        p1 = res.results[2 * b + 1]["part"].reshape(D, 8, SH)
        y = (p0 + p1).transpose(2, 1, 0).reshape(S, D)
        out[b] = y + bo
    return out


# revision 22
# speedup vs baseline: 1.0222x; 1.0109x over previous
"""MultiHeadAttention Trainium2 Bass kernel (8 NeuronCores), v3.

Reference computes (per batch b):
  qp = q @ Wq.T + bq            [S, H*D]   (S=2048, H=8, D=256)
  q_h = qp.reshape(H, S, D)     -- RAW reshape, not split-heads:
        head h <- qp rows [h*256,(h+1)*256), all 2048 cols;
        within head: s2 = ls*8 + g , d  <-> qp[h*256+ls, g*256+d]
  scores_h = q_h @ k_h.T / 16 ; P = softmax ; o_h = P @ v_h
  out[s2, h*256+d] = o_h[s2, d] ;  y = out @ Wo.T + bo

Sharding: core c = (b = c//2, hg = c%2) handles batch b, heads
hg*4..hg*4+4. Within a head we work in the permuted sequence order
s2' = g*256 + ls (softmax is row-wise so a consistent permutation of
rows/cols is exact); host applies the inverse permutation.

v3 design notes (measured on HW):
- f32r matmuls self-load weights (no separate LDWEIGHTS): ~224 ns for
  N=512. bf16/fp8 matmuls pay a SERIAL ldweights (128 cols ~ 107 ns,
  DR 256 cols ~ 213 ns). So everything uses f32r EXCEPT QK^T, where
  fp8e4m3 DoubleRow folds the 256-long contraction into one MM
  (213 LDW + 107 stream = 320/jc vs f32r's 448/jc).
- Q/K projections are written as fp8 (x QA=8) by the DVE bias-add;
  exp() un-scales by 1/(16*QA^2). 2D fp8 tiles + rearranged 3D views
  (the DVE 3D-slice write path corrupts data on HW).
- Output projection is Wo-stationary: yp[o, q] = sum_d WoT[d, oc]
  . o_norm[d, q]; yacc accumulates [o x q']; host un-permutes.
- Emission interleaves QK(ig) with PV(ig-1) per key chunk so the PE
  is not throttled by the exp/activation drain; the next head's K/Q
  projections fill the PV-only steps (keeps HAM warm).
- PSUM: S2 (scores) + A2 (proj q/v + rowsum) + O2 (PV acc) + Y2
  (outproj + proj k) = 8 banks.
"""

import os as _os
import numpy as np

B, S, D, H = 4, 2048, 256, 8
HG = 2            # head groups (cores per batch)
HPG = H // HG     # heads per group = 4
SH = S // H       # seq rows owned by one head = 256
NCORES = 8
QA = 8.0          # fp8 quantization scale for q/k projections
EXP_SCALE = 1.0 / (16.0 * QA * QA)   # undo QA^2, apply 1/sqrt(d_k)

_CACHE = {}


def _build():
    import concourse.bacc as bacc
    import concourse.mybir as mybir
    from concourse.tile import TileContext

    F32 = mybir.dt.float32
    F32R = mybir.dt.float32r
    F8 = mybir.dt.float8e4
    DR = mybir.MatmulPerfMode.DoubleRow
    EXP = mybir.ActivationFunctionType.Exp
    MULT = mybir.AluOpType.mult
    ADD = mybir.AluOpType.add

    nc = bacc.Bacc("TRN2", target_bir_lowering=False)

    SC = HPG * SH  # 1024 seq rows owned by this core

    # ---- DRAM I/O (per-core SPMD) ----
    xqT_d = nc.dram_tensor("xqT", [D, SC], F32R, kind="ExternalInput")
    xkT_d = nc.dram_tensor("xkT", [D, SC], F32R, kind="ExternalInput")
    xvT_d = nc.dram_tensor("xvT", [D, SC], F32R, kind="ExternalInput")
    WqT_d = nc.dram_tensor("WqT", [D, S], F32R, kind="ExternalInput")
    WkT_d = nc.dram_tensor("WkT", [D, S], F32R, kind="ExternalInput")
    WvT_d = nc.dram_tensor("WvT", [D, S], F32R, kind="ExternalInput")
    WoT_d = nc.dram_tensor("WoT", [HPG * D, D], F32R, kind="ExternalInput")
    bqT_d = nc.dram_tensor("bqT", [128, 16], F32, kind="ExternalInput")
    bkT_d = nc.dram_tensor("bkT", [128, 16], F32, kind="ExternalInput")
    bvr_d = nc.dram_tensor("bvr", [1, S], F32, kind="ExternalInput")
    out_d = nc.dram_tensor("part", [D, S], F32, kind="ExternalOutput")

    with TileContext(nc) as tc:
        with nc.allow_low_precision(reason="f32r/fp8 attention"), \
             tc.tile_pool(name="sb", bufs=1) as sb, \
             tc.tile_pool(name="ps", bufs=1, space="PSUM") as ps:

            def sbt(shape, dt, tag, **kw):
                return sb.tile(shape, dt, tag=tag, name=tag, **kw)

            # ---- persistent SBUF tiles ----
            WqT = [sbt([128, S], F32R, f"wq{i}") for i in range(2)]
            WkT = [sbt([128, S], F32R, f"wk{i}") for i in range(2)]
            WvT = [sbt([128, S], F32R, f"wv{i}") for i in range(2)]
            xqT = [sbt([128, SC], F32R, f"xq{i}") for i in range(2)]
            xkT = [sbt([128, SC], F32R, f"xk{i}") for i in range(2)]
            xvT = [sbt([128, SC], F32R, f"xv{i}") for i in range(2)]
            WoT = [sbt([128, D], F32R, f"wo{i}") for i in range(8)]
            bqT = sbt([128, 16], F32, "bqT")
            bkT = sbt([128, 16], F32, "bkT")
            bvr = sbt([1, S], F32, "bvr")
            bvb = sbt([128, S], F32, "bvb")
            # single-slotted projection buffers: head h+1's writes naturally
            # WAR-wait on head h's last reads, which complete a step earlier
            qf8 = [sbt([128, 2 * S], F8, "qf8_0")]
            kf8 = [sbt([128, 2 * S], F8, "kf8_0")]
            qf8_3 = [q[:].rearrange("p (c s) -> p c s", c=2) for q in qf8]
            kf8_3 = [k[:].rearrange("p (c s) -> p c s", c=2) for k in kf8]
            vproj = [[sbt([128, S], F32R, f"vp_{sc}") for sc in range(2)]]
            # 2-ig rotating window for normalized PV output
            o_sb = [sbt([128, 1024], F32R, f"osb{dc}") for dc in range(2)]
            yaccT = [sbt([128, S], F32, f"yacc{oc}") for oc in range(2)]

            # ---- startup DMAs: priority order, round-robin 4 queues ----
            qs = [nc.sync, nc.scalar]
            _qi = [0]

            def dma(dst, src):
                qs[_qi[0] % len(qs)].dma_start(dst, src)
                _qi[0] += 1

            dma(bqT[:], bqT_d[:])
            dma(bkT[:], bkT_d[:])
            dma(bvr[:], bvr_d[:])
            # Q+K paths for head 0 first, interleaved (kchunks follow
            # qchunks almost immediately in the proj emission order)
            for i in range(2):
                dma(xqT[i][:, 0:SH], xqT_d[i * 128:(i + 1) * 128, 0:SH])
            for i in range(2):
                dma(xkT[i][:, 0:SH], xkT_d[i * 128:(i + 1) * 128, 0:SH])
            for q4 in range(4):
                for i in range(2):
                    dma(WqT[i][:, q4 * 512:(q4 + 1) * 512],
                        WqT_d[i * 128:(i + 1) * 128, q4 * 512:(q4 + 1) * 512])
                for i in range(2):
                    dma(WkT[i][:, q4 * 512:(q4 + 1) * 512],
                        WkT_d[i * 128:(i + 1) * 128, q4 * 512:(q4 + 1) * 512])
            # V path for head 0
            for i in range(2):
                dma(xvT[i][:, 0:SH], xvT_d[i * 128:(i + 1) * 128, 0:SH])
            for q4 in range(4):
                for i in range(2):
                    dma(WvT[i][:, q4 * 512:(q4 + 1) * 512],
                        WvT_d[i * 128:(i + 1) * 128, q4 * 512:(q4 + 1) * 512])
            # remaining x columns (heads 1-3), Wo
            for i in range(2):
                dma(xqT[i][:, SH:SC], xqT_d[i * 128:(i + 1) * 128, SH:SC])
                dma(xkT[i][:, SH:SC], xkT_d[i * 128:(i + 1) * 128, SH:SC])
                dma(xvT[i][:, SH:SC], xvT_d[i * 128:(i + 1) * 128, SH:SC])
            for i in range(8):
                dma(WoT[i][:], WoT_d[i * 128:(i + 1) * 128, :])

            nc.gpsimd.partition_broadcast(bvb[:], bvr[:])

            ones_f = sbt([128, 1], F32, "ones_f")
            nc.vector.memset(ones_f[:], 1.0)
            ones = sbt([128, 1], F32R, "ones")
            nc.vector.tensor_copy(ones[:], ones_f[:])

            # HAM pre-warm: dummy matmuls on an (uninitialized) scratch tile
            # keep the PE busy through the startup-DMA window so the clock
            # gate is at 8/8 when the first real matmul issues. Results are
            # never read.
            warm_f = sbt([128, 512], F32, "warm_f")
            nc.vector.memset(warm_f[:], 0.0)
            warm_in = sbt([128, 512], F32R, "warm_in")
            nc.vector.tensor_copy(warm_in[:], warm_f[:])
            for _ in range(16):
                wp = ps.tile([128, 512], F32, tag="S", bufs=2, name="wp")
                nc.tensor.matmul(wp[:], warm_in[:, 0:128], warm_in[:],
                                 start=True, stop=True)

            # ---- emitters ----
            P = {}       # (ig, jc) -> pt tile (transient per head)
            STATE = {}   # ig -> (rs, o_ps pair)

            def emit_qchunk(t, ec):
                pq = ps.tile([128, 512], F32, tag="A", bufs=2, name="pq")
                for dc in range(2):
                    nc.tensor.matmul(
                        pq[:, :SH],
                        WqT[dc][:, ec * 128:(ec + 1) * 128],
                        xqT[dc][:, t * SH:(t + 1) * SH],
                        start=(dc == 0), stop=(dc == 1))
                g, dct = divmod(ec, 2)
                nc.vector.tensor_scalar(
                    out=qf8[0][:, dct * S + g * SH:dct * S + (g + 1) * SH],
                    in0=pq[:, :SH], scalar1=QA,
                    scalar2=bqT[:, ec:ec + 1], op0=MULT, op1=ADD)

            def emit_kchunk(t, ec):
                pk = ps.tile([128, 512], F32, tag="Y", bufs=2, name="pk")
                for dc in range(2):
                    nc.tensor.matmul(
                        pk[:, :SH],
                        WkT[dc][:, ec * 128:(ec + 1) * 128],
                        xkT[dc][:, t * SH:(t + 1) * SH],
                        start=(dc == 0), stop=(dc == 1))
                g, dct = divmod(ec, 2)
                nc.vector.tensor_scalar(
                    out=kf8[0][:, dct * S + g * SH:dct * S + (g + 1) * SH],
                    in0=pk[:, :SH], scalar1=QA,
                    scalar2=bkT[:, ec:ec + 1], op0=MULT, op1=ADD)

            def emit_vchunk(t, i):
                sc, ng = divmod(i, 4)
                pv = ps.tile([128, 512], F32, tag="A", bufs=2, name="pv")
                for dc in range(2):
                    nc.tensor.matmul(
                        pv[:],
                        xvT[dc][:, t * SH + sc * 128:t * SH + (sc + 1) * 128],
                        WvT[dc][:, ng * 512:(ng + 1) * 512],
                        start=(dc == 0), stop=(dc == 1))
                nc.vector.tensor_add(
                    vproj[0][sc][:, ng * 512:(ng + 1) * 512], pv[:],
                    bvb[:, ng * 512:(ng + 1) * 512])

            def emit_qk(h, ig, jc):
                sp = ps.tile([128, 512], F32, tag="S", bufs=2, name="sp")
                nc.tensor.matmul(
                    sp[:],
                    kf8_3[0][:, :, jc * 128:(jc + 1) * 128],
                    qf8_3[0][:, :, ig * 512:(ig + 1) * 512],
                    start=True, stop=True, perf_mode=DR)
                pt = sb.tile([128, 512], F32R, tag="p", bufs=20, name="pt")
                nc.scalar.activation(pt[:], sp[:], EXP, scale=EXP_SCALE)
                P[(ig, jc)] = pt

            def emit_pv(h, ig, jc):
                if jc == 0:
                    rs = ps.tile([128, 512], F32, tag="A", bufs=2, name="rs")
                    o_ps = [ps.tile([128, 512], F32, tag="O", bufs=2,
                                    name=f"o{dc}") for dc in range(2)]
                    STATE[ig] = (rs, o_ps)
                rs, o_ps = STATE[ig]
                g, half = divmod(jc, 2)
                pt = P.pop((ig, jc))
                nc.tensor.matmul(
                    rs[0:1, :], ones[:], pt[:],
                    start=(jc == 0), stop=(jc == 15), skip_group_check=True)
                for dc in range(2):
                    nc.tensor.matmul(
                        o_ps[dc][:],
                        vproj[0][half][:, g * SH + dc * 128:g * SH + (dc + 1) * 128],
                        pt[:],
                        start=(jc == 0), stop=(jc == 15), skip_group_check=True)

            def emit_norm(h, ig):
                rs, o_ps = STATE.pop(ig)
                rcp = sb.tile([1, 512], F32, tag="rcp", bufs=2, name="rcp")
                nc.vector.reciprocal_approx_fast(out=rcp[:], in_=rs[0:1, :])
                bc = sb.tile([128, 512], F32, tag="bc", bufs=2, name="bc")
                nc.gpsimd.partition_broadcast(bc[:], rcp[:])
                w = (ig % 2) * 512
                for dc in range(2):
                    nc.vector.tensor_mul(
                        o_sb[dc][:, w:w + 512], o_ps[dc][:], bc[:])

            def emit_outproj(h, ig, ocs=(0, 1)):
                icol = ig * 512
                w = (ig % 2) * 512
                for oc in ocs:
                    yp = ps.tile([128, 512], F32, tag="Y", bufs=2, name="yp")
                    for dc in range(2):
                        nc.tensor.matmul(
                            yp[:],
                            WoT[h * 2 + dc][:, oc * 128:(oc + 1) * 128],
                            o_sb[dc][:, w:w + 512],
                            start=(dc == 0), stop=(dc == 1))
                    if h == 0:
                        nc.vector.tensor_copy(yaccT[oc][:, icol:icol + 512], yp[:])
                    else:
                        nc.vector.tensor_add(
                            yaccT[oc][:, icol:icol + 512],
                            yaccT[oc][:, icol:icol + 512], yp[:])
                        if h == HPG - 1:
                            qd = nc.sync if oc == 0 else nc.scalar
                            qd.dma_start(
                                out_d[oc * 128:(oc + 1) * 128, icol:icol + 512],
                                yaccT[oc][:, icol:icol + 512])

            # ---- head-0 projections (DMA-gated warmup) ----
            for ec in range(4):
                emit_qchunk(0, ec)
            for ec in range(16):
                emit_kchunk(0, ec)
            for ec in range(4, 16):
                emit_qchunk(0, ec)

            # ---- pipelined head loop ----
            # S0: QK(ig0) + V-proj fillers
            # S1-3: QK(ig) interleaved with PV(ig-1) [+ outproj(ig-2) @jc6]
            # S4: PV(ig3) + K-proj(h+1) fillers [+ outproj(ig2) @jc6]
            # S5: Q-proj(h+1) + outproj(ig3)
            for h in range(HPG):
                # S0
                for jc in range(16):
                    if jc % 2 == 0:
                        emit_vchunk(h, jc // 2)
                    emit_qk(h, 0, jc)
                # S1..S3
                for ig in range(1, 4):
                    for jc in range(16):
                        emit_qk(h, ig, jc)
                        if ig >= 2 and jc == 3:
                            emit_outproj(h, ig - 2, (0,))
                        if ig >= 2 and jc == 9:
                            emit_outproj(h, ig - 2, (1,))
                        emit_pv(h, ig - 1, jc)
                    emit_norm(h, ig - 1)
                # S4
                for jc in range(16):
                    if h < HPG - 1:
                        emit_kchunk(h + 1, jc)
                    if jc == 3:
                        emit_outproj(h, 2, (0,))
                    if jc == 9:
                        emit_outproj(h, 2, (1,))
                    emit_pv(h, 3, jc)
                emit_norm(h, 3)
                # S5
                if h < HPG - 1:
                    for ec in range(16):
                        if ec == 4:
                            emit_outproj(h, 3)
                        emit_qchunk(h + 1, ec)
                else:
                    emit_outproj(h, 3)

    nc.finalize()
    return nc


def _get_nc():
    if "nc" not in _CACHE:
        _CACHE["nc"] = _build()
    return _CACHE["nc"]


def _prep_inputs(query, key, values, Wq, bq, Wk, bk, Wv, bv, Wo, bo):
    f32 = np.float32
    query = np.asarray(query, f32)
    key = np.asarray(key, f32)
    values = np.asarray(values, f32)
    WqT = np.ascontiguousarray(np.asarray(Wq, f32).T)
    WkT = np.ascontiguousarray(np.asarray(Wk, f32).T)
    WvT = np.ascontiguousarray(np.asarray(Wv, f32).T)
    WoT = np.ascontiguousarray(np.asarray(Wo, f32).T)
    bqT = np.ascontiguousarray((np.asarray(bq, f32) * QA).reshape(16, 128).T)
    bkT = np.ascontiguousarray((np.asarray(bk, f32) * QA).reshape(16, 128).T)
    bvr = np.ascontiguousarray(np.asarray(bv, f32).reshape(1, S))

    in_maps = []
    for c in range(NCORES):
        b, hg = divmod(c, HG)
        rows = slice(hg * HPG * SH, (hg + 1) * HPG * SH)
        in_maps.append({
            "xqT": np.ascontiguousarray(query[b, rows, :].T),
            "xkT": np.ascontiguousarray(key[b, rows, :].T),
            "xvT": np.ascontiguousarray(values[b, rows, :].T),
            "WqT": WqT, "WkT": WkT, "WvT": WvT,
            "WoT": np.ascontiguousarray(WoT[hg * HPG * D:(hg + 1) * HPG * D, :]),
            "bqT": bqT, "bkT": bkT, "bvr": bvr,
        })
    return in_maps


def _enable_tracing_shims():
    """Best-effort: make trace=True survivable in environments where the
    image's antenv lacks axon_hooks and artifact upload has no network."""
    import sys
    import types
    try:
        import antenv.axon_hooks  # noqa: F401
    except Exception:
        try:
            from trn_agent_boot.trn_boot import _ntff_profile_via_ctypes
            hook = _ntff_profile_via_ctypes("/opt/axon/libaxon_pjrt.so")
            mod = types.ModuleType("antenv.axon_hooks")
            mod.get_axon_ntff_profile_hook = lambda: hook
            mod.set_axon_ntff_profile_hook = lambda h: None
            sys.modules["antenv.axon_hooks"] = mod
            import antenv
            antenv.axon_hooks = mod
        except Exception:
            pass
    try:
        import concourse.bass_utils as bu
        from concourse._compat import FishPath
        FishPath.bucket_root()
    except Exception:
        try:
            bu.upload_artifacts = lambda tmpdir: f"local://{tmpdir}"
        except Exception:
            pass


def kernel(**inputs):
    import os
    from concourse.bass_utils import run_bass_kernel_spmd

    nc = _get_nc()
    in_maps = _prep_inputs(**inputs)
    trace = bool(int(os.environ.get("KERNEL_TRACE", "0")))
    if trace or os.environ.get("BASS_TRACE"):
        _enable_tracing_shims()
    res = run_bass_kernel_spmd(nc, in_maps, core_ids=list(range(NCORES)),
                               trace=trace)
    _CACHE["last_result"] = res

    bo = np.asarray(inputs["bo"], np.float32)
    out = np.empty((B, S, D), np.float32)
    for b in range(B):
        # part[o, q'] with q' = g*256 + ls ; true s2 = ls*8 + g
        p0 = res.results[2 * b]["part"].reshape(D, 8, SH)
        p1 = res.results[2 * b + 1]["part"].reshape(D, 8, SH)
        y = (p0 + p1).transpose(2, 1, 0).reshape(S, D)
        out[b] = y + bo
    return out
